# revision 14
# baseline (speedup 1.0000x reference)
"""Trainium2 Bass kernel for the CMDF block (dense_cnn).

Contract: kernel(**inputs) takes the FULL unsharded inputs (B=8, C=128,
H=W=64) and returns the FULL (8, 128, 64, 64) float32 output.

Sharding: data-parallel over batch — core b computes batch element b.
All weights are replicated (host-side prepacked into matmul layouts).

Math per batch element (see reference):
  Xs   = depthwise3x3(X2, static_w)
  ctx  = relu(w2 @ (w1 @ mean_hw([Xs; Y2])))
  cf   = (w3 @ ctx).reshape(C, 9)          # per-channel dynamic filter
  sf   = ws @ [Xs; Y2]                     # (9, H, W) spatial filter
  dyn  = sum_k shift_k(X2) * (cf[:, k] + sf[k])
  out  = wf[:, :C] @ Xs + wf[:, C:] @ dyn

Kernel strategy (channels on partitions, pixels on the free dim):
  - Xs via 9 accumulating PE matmuls with diag(sw[:, k]) weights over a
    zero-padded X held in SBUF. All large matmuls run in fp32r (full-rate
    fp32 mode, 11-bit mantissa); operands are pre-rounded on the host or
    rounded on-chip by their producing ACT/DVE instruction.
  - sf via matmuls with M=105 (ws replicated into 4 row-groups so the
    per-tap partition-broadcast matmuls can be row-tiled).
  - per tap k: broadcast sf[k] to 128 partitions with a 0/1 "selector"
    matmul, then ONE fused DVE op P_k = (sf_bc + cf[:,k]) * shift_k(X),
    then an accumulating matmul out += wfbT.T @ P_k. The sum over taps
    happens inside the final conv's PSUM accumulation.
"""

import numpy as np

import concourse.bass as bass
import concourse.tile as tile
import concourse.mybir as mybir
from concourse.bass_utils import run_bass_kernel_spmd

B, C, H, W, K = 8, 128, 64, 64, 3
HW = H * W            # 4096
PH, PW = H + 2, W + 2  # 66, 66 padded
NST = 4               # super-tiles over rows
ROWS = H // NST       # 16 image rows per super-tile
STN = ROWS * W        # 1024 pixels per super-tile (2 PSUM banks)
NT = K * K            # 9 taps
MREP = 3 * 32 + NT    # 105: ws replicated at partition groups 0,32,64,96

F32 = mybir.dt.float32
F32R = mybir.dt.float32r
ADD = mybir.AluOpType.add
MULT = mybir.AluOpType.mult
AX = mybir.AxisListType
ACT_COPY = mybir.ActivationFunctionType.Copy
ACT_RELU = mybir.ActivationFunctionType.Relu

_CACHE = {}


def round_f32r(a):
    """Round fp32 to fp32r (RNE at mantissa bit 12) — matches the
    walrus cast_fp32_to_fp32r used by the FP32r matmul datapath."""
    u = np.ascontiguousarray(a, dtype=np.float32).view(np.uint32).astype(np.uint64)
    r = ((u + 0x7FF + ((u >> 12) & 1)) & 0xFFFFF000).astype(np.uint32)
    return r.view(np.float32).reshape(np.asarray(a).shape)


BF16 = mybir.dt.bfloat16


def _absorb(nc, dep_elem, ps_elem):
    """Tiny bf16 matmul that reads one element of `dep_elem` and writes a
    junk element of `ps_elem` (later overwritten by a start=True group).
    Purpose: acquire the semaphore wait on dep_elem's producer on a plain
    (non-fused) matmul, so the following fused f32r matmul — which can
    embed only ONE sem wait — doesn't need two."""
    lh = dep_elem.bitcast(BF16)
    nc.tensor.matmul(ps_elem, lh[:, 0:1], lh[:, 0:1], start=True, stop=True)


def _split_multiwaits(nc):
    """walrus codegen in this toolchain accepts only ONE embedded sem wait
    per instruction. Hoist excess waits onto same-engine NoOps placed
    immediately before the instruction (engines execute in order, so the
    blocking behavior is identical)."""
    ctr = 0
    for fn in nc.m.functions:
        for blk in fn.blocks:
            insts = blk.instructions
            out = []
            for inst in insts:
                si = inst.sync_info
                waits = list(si.on_wait) if si is not None and si.on_wait else []
                if len(waits) > 1:
                    for w in waits[:-1]:
                        ctr += 1
                        out.append(mybir.InstNoOp(
                            name=f"I-wsplit-{ctr}",
                            engine=inst.engine,
                            ins=[], outs=[],
                            sync_info=mybir.SyncInfo(
                                on_wait=[w], on_update=[]),
                        ))
                    inst.sync_info = mybir.SyncInfo(
                        on_wait=[waits[-1]],
                        on_update=list(si.on_update) if si.on_update else [],
                    )
                out.append(inst)
            blk.instructions = out


def _build_bass():
    nc = bass.Bass("TRN2", target_bir_lowering=False, debug=False)

    # single input pack: xpad | y2 | dsw | wsa | wsb | wfa | wfb | bct | w1ab | w2t+w3t
    # one DMA -> one producer proc -> every consumer needs at most one wait
    WR_COLS = NT * C + MREP + MREP + C + C + NT * C  # 2770
    PK_COLS = PH * PW + HW + WR_COLS + 2 * 64 + (64 + NT * C)
    pk = nc.dram_tensor("pk", [C, PK_COLS], F32R, kind="ExternalInput").ap()
    ob = nc.dram_tensor("ob", [C, H, W], F32, kind="ExternalOutput").ap()

    with tile.TileContext(nc) as tc:
        with tc.tile_pool(name="singles", bufs=1) as S:
            stg = S.tile([C, PK_COLS], F32R)
            o = 0
            xpad = stg[:, o : o + PH * PW].rearrange(
                "p (h w) -> p h w", w=PW); o += PH * PW
            y2 = stg[:, o : o + HW]; o += HW
            t_dsw = stg[:, o : o + NT * C]; o += NT * C
            t_wsa = stg[:, o : o + MREP]; o += MREP
            t_wsb = stg[:, o : o + MREP]; o += MREP
            t_wfa = stg[:, o : o + C]; o += C
            t_wfb = stg[:, o : o + C]; o += C
            t_bct = stg[:, o : o + NT * C]; o += NT * C
            t_w1a = stg[:, o : o + 64].bitcast(F32); o += 64
            t_w1b = stg[:, o : o + 64].bitcast(F32); o += 64
            t_w2t = stg[0:64, o : o + 64].bitcast(F32); o += 64
            t_w3t = stg[0:64, o : o + NT * C].bitcast(F32); o += NT * C
            assert o == PK_COLS
            xs = S.tile([C, HW], F32R)
            sfs = S.tile([MREP, HW], F32R)

            xs_parts = S.tile([C, NST], F32)
            y2sum = S.tile([C, 1], F32)
            xs_sum = S.tile([C, 1], F32)
            mxs = S.tile([C, 1], F32)
            my2 = S.tile([C, 1], F32)
            ctx1 = S.tile([64, 1], F32)
            ctx2 = S.tile([64, 1], F32)
            cfsb = S.tile([C, NT], F32)

            nc.sync.dma_start(out=stg, in_=pk)

            # mean(Y2) ingredient — DVE is idle during phase A
            nc.vector.tensor_reduce(out=y2sum, in_=y2, axis=AX.X, op=ADD)

            # ---------- phase A: Xs (static depthwise) + sf ----------
            with tc.tile_pool(name="psA", bufs=2, space="PSUM") as psA, \
                 tc.tile_pool(name="psSF", bufs=1, space="PSUM") as psSF:
                for t in range(NST):
                    xs_ps = psA.tile([C, 2, 512], F32, tag="xs_ps")
                    for h in range(2):
                        for k in range(NT):
                            dh, dw = divmod(k, 3)
                            r0 = 16 * t + 8 * h + dh
                            rhs = xpad[:, r0 : r0 + 8, dw : dw + W]
                            nc.tensor.matmul(
                                xs_ps[:, h, :],
                                t_dsw[:, k * C : (k + 1) * C],
                                rhs,
                                start=(k == 0),
                                stop=(k == NT - 1),
                            )
                    nc.scalar.activation(
                        out=xs[:, t * STN : (t + 1) * STN],
                        in_=xs_ps,
                        func=ACT_COPY,
                        accum_out=xs_parts[:, t : t + 1],
                    )
                    sf_ps = psSF.tile([MREP, 2, 512], F32, tag="sf_ps")
                    _absorb(nc, xs[0:1, t * STN : t * STN + 1],
                            sf_ps[0:1, 0, 0:1])
                    for h in range(2):
                        c0 = t * STN + h * 512
                        nc.tensor.matmul(
                            sf_ps[:, h, :],
                            t_wsa,
                            xs[:, c0 : c0 + 512],
                            start=True,
                            stop=False,
                        )
                        nc.tensor.matmul(
                            sf_ps[:, h, :],
                            t_wsb,
                            y2[:, c0 : c0 + 512],
                            start=False,
                            stop=True,
                        )
                    nc.scalar.copy(
                        out=sfs[:, t * STN : (t + 1) * STN], in_=sf_ps
                    )

            # ---------- phase B: context branch -> cf ----------
            with tc.tile_pool(name="psCtx", bufs=1, space="PSUM") as psX:
                nc.vector.tensor_reduce(out=xs_sum, in_=xs_parts, axis=AX.X, op=ADD)
                nc.scalar.mul(out=mxs, in_=xs_sum, mul=1.0 / HW)
                nc.scalar.mul(out=my2, in_=y2sum, mul=1.0 / HW)

                ctx1_ps = psX.tile([64, 1], F32, tag="ctx1")
                _absorb(nc, mxs[0:1, 0:1], ctx1_ps[0:1, 0:1])
                nc.tensor.matmul(ctx1_ps, t_w1a, mxs, start=True, stop=False)
                nc.tensor.matmul(ctx1_ps, t_w1b, my2, start=False, stop=True)
                nc.scalar.copy(out=ctx1, in_=ctx1_ps)

                ctx2_ps = psX.tile([64, 1], F32, tag="ctx2")
                nc.tensor.matmul(ctx2_ps, t_w2t, ctx1, start=True, stop=True)
                nc.scalar.activation(out=ctx2, in_=ctx2_ps, func=ACT_RELU)

                cf_ps = psX.tile([C, NT], F32, tag="cf")
                for k in range(NT):
                    nc.tensor.matmul(
                        cf_ps[:, k : k + 1], t_w3t[:, k * C : (k + 1) * C],
                        ctx2, start=True, stop=True,
                    )
                nc.scalar.copy(out=cfsb, in_=cf_ps)

            # ---------- phase C: dynamic filter + fusion conv ----------
            with tc.tile_pool(name="psBC", bufs=2, space="PSUM") as psBC, \
                 tc.tile_pool(name="psOut", bufs=2, space="PSUM") as psO, \
                 tc.tile_pool(name="pP", bufs=3) as pP, \
                 tc.tile_pool(name="pOsb", bufs=2) as pOsb:
                for t in range(NST):
                    out_ps = psO.tile([C, 2, 8, W], F32, tag="out_ps")
                    _absorb(nc, xs[0:1, t * STN : t * STN + 1],
                            out_ps[0:1, 0, 0, 0:1])
                    for h in range(2):
                        c0 = t * STN + h * 512
                        nc.tensor.matmul(
                            out_ps[:, h],
                            t_wfa,
                            xs[:, c0 : c0 + 512],
                            start=True,
                            stop=False,
                        )
                    for k in range(NT):
                        g = k % 2
                        bc_ps = psBC.tile([C, ROWS, W], F32, tag="bc")
                        if k == 0:
                            _absorb(nc, sfs[0:1, t * STN : t * STN + 1],
                                    bc_ps[0:1, 0, 0:1])
                        for h in range(2):
                            c0 = t * STN + h * 512
                            nc.tensor.matmul(
                                bc_ps[:, 8 * h : 8 * h + 8, :],
                                t_bct[32 * g : 32 * g + NT,
                                      k * C : (k + 1) * C],
                                sfs[32 * g : 32 * g + NT, c0 : c0 + 512],
                                start=True,
                                stop=True,
                                tile_position=(32 * g, 0),
                            )
                        dh, dw = divmod(k, 3)
                        p_sb = pP.tile([C, ROWS, W], F32R, tag="p")
                        nc.vector.scalar_tensor_tensor(
                            out=p_sb,
                            in0=bc_ps,
                            scalar=cfsb[:, k : k + 1],
                            in1=xpad[:, 16 * t + dh : 16 * t + dh + ROWS,
                                     dw : dw + W],
                            op0=ADD,
                            op1=MULT,
                        )
                        for h in range(2):
                            nc.tensor.matmul(
                                out_ps[:, h],
                                t_wfb,
                                p_sb[:, 8 * h : 8 * h + 8, :],
                                start=False,
                                stop=(k == NT - 1),
                            )
                    o_sb = pOsb.tile([C, 2, 8, W], F32, tag="osb")
                    nc.scalar.copy(out=o_sb, in_=out_ps)
                    nc.sync.dma_start(
                        out=ob[:, 16 * t : 16 * t + 16, :],
                        in_=o_sb.rearrange("c b r w -> c (b r) w"),
                    )
    _split_multiwaits(nc)
    return nc


def _prep_weights(static_w, w1, w2, w3, ws, wf):
    """Repack the tiny weights into the SBUF layouts the kernel expects."""
    f = np.float32
    sw = np.ascontiguousarray(static_w.reshape(C, NT), dtype=f)

    dsw = np.zeros((C, NT * C), dtype=f)
    for k in range(NT):
        dsw[np.arange(C), k * C + np.arange(C)] = sw[:, k]

    wsa = np.zeros((C, MREP), dtype=f)
    wsb = np.zeros((C, MREP), dtype=f)
    for g in range(4):
        for k in range(NT):
            wsa[:, 32 * g + k] = ws[k, :C]
            wsb[:, 32 * g + k] = ws[k, C:]

    bct = np.zeros((C, NT * C), dtype=f)
    for g in range(4):
        for k in range(NT):
            bct[32 * g + k, k * C : (k + 1) * C] = 1.0

    wfa = np.ascontiguousarray(wf[:, :C].T, dtype=f)
    wfb = np.ascontiguousarray(wf[:, C:].T, dtype=f)
    wr = round_f32r(
        np.concatenate([dsw, wsa, wsb, wfa, wfb, bct], axis=1)
    )
    wfp = np.concatenate(
        [np.ascontiguousarray(w1[:, :C].T, dtype=f),
         np.ascontiguousarray(w1[:, C:].T, dtype=f)], axis=1
    )
    w3t = np.ascontiguousarray(
        w3.reshape(C, NT, 64).transpose(2, 1, 0), dtype=f
    ).reshape(64, NT * C)
    wg64 = np.concatenate(
        [np.ascontiguousarray(w2.T, dtype=f), w3t], axis=1
    )
    wg = np.zeros((C, wg64.shape[1]), dtype=f)
    wg[:64] = wg64
    return np.concatenate([wr, wfp, wg], axis=1)


def make_in_maps(X2, Y2, static_w, w1, w2, w3, ws, wf):
    wpack = _prep_weights(
        np.asarray(static_w), np.asarray(w1), np.asarray(w2),
        np.asarray(w3), np.asarray(ws), np.asarray(wf),
    )
    X2 = np.asarray(X2)
    Y2 = np.asarray(Y2)
    xpad_all = np.zeros((B, C, PH, PW), dtype=np.float32)
    xpad_all[:, :, 1 : H + 1, 1 : W + 1] = X2
    xpad_all = round_f32r(xpad_all).reshape(B, C, PH * PW)
    y2_all = round_f32r(Y2.reshape(B, C, HW))
    in_maps = []
    for b in range(B):
        m = {"pk": np.ascontiguousarray(np.concatenate(
            [xpad_all[b], y2_all[b], wpack], axis=1))}
        in_maps.append(m)
    return in_maps


def get_nc():
    if "nc" not in _CACHE:
        _CACHE["nc"] = _build_bass()
    return _CACHE["nc"]


def kernel(X2, Y2, static_w, w1, w2, w3, ws, wf):
    nc = get_nc()
    in_maps = make_in_maps(
        np.asarray(X2), np.asarray(Y2), static_w, w1, w2, w3, ws, wf
    )
    res = run_bass_kernel_spmd(nc, in_maps, core_ids=list(range(B)))
    out = np.stack([r["ob"] for r in res.results]).astype(np.float32)
    return out


# revision 15
# speedup vs baseline: 18038.9382x; 18038.9382x over previous
"""Trainium2 Bass kernel for the CMDF block (dense_cnn).

Contract: kernel(**inputs) takes the FULL unsharded inputs (B=8, C=128,
H=W=64) and returns the FULL (8, 128, 64, 64) float32 output.

Sharding: data-parallel over batch — core b computes batch element b.
All weights are replicated (host-side prepacked into matmul layouts).

Math per batch element (see reference):
  Xs   = depthwise3x3(X2, static_w)
  ctx  = relu(w2 @ (w1 @ mean_hw([Xs; Y2])))
  cf   = (w3 @ ctx).reshape(C, 9)          # per-channel dynamic filter
  sf   = ws @ [Xs; Y2]                     # (9, H, W) spatial filter
  dyn  = sum_k shift_k(X2) * (cf[:, k] + sf[k])
  out  = wf[:, :C] @ Xs + wf[:, C:] @ dyn

Kernel strategy (channels on partitions, pixels on the free dim):
  - Xs via 9 accumulating PE matmuls with diag(sw[:, k]) weights over a
    zero-padded X held in SBUF. All large matmuls run in fp32r (full-rate
    fp32 mode, 11-bit mantissa); operands are pre-rounded on the host or
    rounded on-chip by their producing ACT/DVE instruction.
  - sf via matmuls with M=105 (ws replicated into 4 row-groups so the
    per-tap partition-broadcast matmuls can be row-tiled).
  - per tap k: broadcast sf[k] to 128 partitions with a 0/1 "selector"
    matmul, then ONE fused DVE op P_k = (sf_bc + cf[:,k]) * shift_k(X),
    then an accumulating matmul out += wfbT.T @ P_k. The sum over taps
    happens inside the final conv's PSUM accumulation.
"""

import numpy as np

import concourse.bass as bass
import concourse.tile as tile
import concourse.mybir as mybir
from concourse.bass_utils import run_bass_kernel_spmd

B, C, H, W, K = 8, 128, 64, 64, 3
HW = H * W            # 4096
PH, PW = H + 2, W + 2  # 66, 66 padded
NST = 4               # super-tiles over rows
ROWS = H // NST       # 16 image rows per super-tile
STN = ROWS * W        # 1024 pixels per super-tile (2 PSUM banks)
NT = K * K            # 9 taps
MREP = 3 * 32 + NT    # 105: ws replicated at partition groups 0,32,64,96

F32 = mybir.dt.float32
F32R = mybir.dt.float32r
ADD = mybir.AluOpType.add
MULT = mybir.AluOpType.mult
AX = mybir.AxisListType
ACT_COPY = mybir.ActivationFunctionType.Copy
ACT_RELU = mybir.ActivationFunctionType.Relu

_CACHE = {}


def round_f32r(a):
    """Round fp32 to fp32r (RNE at mantissa bit 12) — matches the
    walrus cast_fp32_to_fp32r used by the FP32r matmul datapath."""
    u = np.ascontiguousarray(a, dtype=np.float32).view(np.uint32).astype(np.uint64)
    r = ((u + 0x7FF + ((u >> 12) & 1)) & 0xFFFFF000).astype(np.uint32)
    return r.view(np.float32).reshape(np.asarray(a).shape)


BF16 = mybir.dt.bfloat16


def _absorb(nc, dep_elem, ps_elem):
    """Tiny bf16 matmul that reads one element of `dep_elem` and writes a
    junk element of `ps_elem` (later overwritten by a start=True group).
    Purpose: acquire the semaphore wait on dep_elem's producer on a plain
    (non-fused) matmul, so the following fused f32r matmul — which can
    embed only ONE sem wait — doesn't need two."""
    lh = dep_elem.bitcast(BF16)
    nc.tensor.matmul(ps_elem, lh[:, 0:1], lh[:, 0:1], start=True, stop=True)


def _split_multiwaits(nc):
    """walrus codegen in this toolchain accepts only ONE embedded sem wait
    per instruction. Hoist excess waits onto same-engine NoOps placed
    immediately before the instruction (engines execute in order, so the
    blocking behavior is identical)."""
    ctr = 0
    for fn in nc.m.functions:
        for blk in fn.blocks:
            insts = blk.instructions
            out = []
            for inst in insts:
                si = inst.sync_info
                waits = list(si.on_wait) if si is not None and si.on_wait else []
                if len(waits) > 1:
                    for w in waits[:-1]:
                        ctr += 1
                        out.append(mybir.InstNoOp(
                            name=f"I-wsplit-{ctr}",
                            engine=inst.engine,
                            ins=[], outs=[],
                            sync_info=mybir.SyncInfo(
                                on_wait=[w], on_update=[]),
                        ))
                    inst.sync_info = mybir.SyncInfo(
                        on_wait=[waits[-1]],
                        on_update=list(si.on_update) if si.on_update else [],
                    )
                out.append(inst)
            blk.instructions = out


def _build_bass():
    nc = bass.Bass("TRN2", target_bir_lowering=False, debug=False)

    # single input pack: xpad | y2 | dsw | wsa | wsb | wfa | wfb | bct | w1ab | w2t+w3t
    # one DMA -> one producer proc -> every consumer needs at most one wait
    WR_COLS = NT * C + MREP + MREP + C + C + NT * C  # 2770
    PK_COLS = PH * PW + HW + WR_COLS + 2 * 64 + (64 + NT * C)
    pk = nc.dram_tensor("pk", [C, PK_COLS], F32R, kind="ExternalInput").ap()
    ob = nc.dram_tensor("ob", [C, H, W], F32, kind="ExternalOutput").ap()

    with tile.TileContext(nc) as tc:
        with tc.tile_pool(name="singles", bufs=1) as S:
            stg = S.tile([C, PK_COLS], F32R)
            o = 0
            xpad = stg[:, o : o + PH * PW].rearrange(
                "p (h w) -> p h w", w=PW); o += PH * PW
            y2 = stg[:, o : o + HW]; o += HW
            t_dsw = stg[:, o : o + NT * C]; o += NT * C
            t_wsa = stg[:, o : o + MREP]; o += MREP
            t_wsb = stg[:, o : o + MREP]; o += MREP
            t_wfa = stg[:, o : o + C]; o += C
            t_wfb = stg[:, o : o + C]; o += C
            t_bct = stg[:, o : o + NT * C]; o += NT * C
            t_w1a = stg[:, o : o + 64].bitcast(F32); o += 64
            t_w1b = stg[:, o : o + 64].bitcast(F32); o += 64
            t_w2t = stg[0:64, o : o + 64].bitcast(F32); o += 64
            t_w3t = stg[0:64, o : o + NT * C].bitcast(F32); o += NT * C
            assert o == PK_COLS
            xs = S.tile([C, HW], F32R)
            sfs = S.tile([MREP, HW], F32R)

            xs_parts = S.tile([C, NST], F32)
            y2sum = S.tile([C, 1], F32)
            xs_sum = S.tile([C, 1], F32)
            mxs = S.tile([C, 1], F32)
            my2 = S.tile([C, 1], F32)
            ctx1 = S.tile([64, 1], F32)
            ctx2 = S.tile([64, 1], F32)
            cfsb = S.tile([C, NT], F32)

            # split the input load across DMA queues (the wait-splitter
            # pass makes multi-producer fan-in legal)
            A = PH * PW
            Bc = PH * PW + HW
            nc.sync.dma_start(out=stg[:, 0:A], in_=pk[:, 0:A])
            nc.sync.dma_start(out=stg[:, A:Bc], in_=pk[:, A:Bc])
            nc.sync.dma_start(out=stg[:, Bc:], in_=pk[:, Bc:])

            # mean(Y2) ingredient — DVE is idle during phase A
            nc.vector.tensor_reduce(out=y2sum, in_=y2, axis=AX.X, op=ADD)

            # ---------- phase A: Xs (static depthwise) + sf ----------
            with tc.tile_pool(name="psA", bufs=2, space="PSUM") as psA, \
                 tc.tile_pool(name="psSF", bufs=2, space="PSUM") as psSF:
                for t in range(NST):
                    xs_ps = psA.tile([C, 2, 512], F32, tag="xs_ps")
                    for h in range(2):
                        for k in range(NT):
                            dh, dw = divmod(k, 3)
                            r0 = 16 * t + 8 * h + dh
                            rhs = xpad[:, r0 : r0 + 8, dw : dw + W]
                            nc.tensor.matmul(
                                xs_ps[:, h, :],
                                t_dsw[:, k * C : (k + 1) * C],
                                rhs,
                                start=(k == 0),
                                stop=(k == NT - 1),
                            )
                    nc.scalar.activation(
                        out=xs[:, t * STN : (t + 1) * STN],
                        in_=xs_ps,
                        func=ACT_COPY,
                        accum_out=xs_parts[:, t : t + 1],
                    )
                    sf_ps = psSF.tile([MREP, 2, 512], F32, tag="sf_ps")
                    _absorb(nc, xs[0:1, t * STN : t * STN + 1],
                            sf_ps[0:1, 0, 0:1])
                    for h in range(2):
                        c0 = t * STN + h * 512
                        nc.tensor.matmul(
                            sf_ps[:, h, :],
                            t_wsa,
                            xs[:, c0 : c0 + 512],
                            start=True,
                            stop=False,
                        )
                        nc.tensor.matmul(
                            sf_ps[:, h, :],
                            t_wsb,
                            y2[:, c0 : c0 + 512],
                            start=False,
                            stop=True,
                        )
                    nc.scalar.copy(
                        out=sfs[:, t * STN : (t + 1) * STN], in_=sf_ps
                    )

            # ---------- phase B: context branch -> cf ----------
            with tc.tile_pool(name="psCtx", bufs=1, space="PSUM") as psX:
                nc.vector.tensor_reduce(out=xs_sum, in_=xs_parts, axis=AX.X, op=ADD)
                nc.scalar.mul(out=mxs, in_=xs_sum, mul=1.0 / HW)
                nc.scalar.mul(out=my2, in_=y2sum, mul=1.0 / HW)

                ctx1_ps = psX.tile([64, 1], F32, tag="ctx1")
                _absorb(nc, mxs[0:1, 0:1], ctx1_ps[0:1, 0:1])
                nc.tensor.matmul(ctx1_ps, t_w1a, mxs, start=True, stop=False)
                nc.tensor.matmul(ctx1_ps, t_w1b, my2, start=False, stop=True)
                nc.scalar.copy(out=ctx1, in_=ctx1_ps)

                ctx2_ps = psX.tile([64, 1], F32, tag="ctx2")
                nc.tensor.matmul(ctx2_ps, t_w2t, ctx1, start=True, stop=True)
                nc.scalar.activation(out=ctx2, in_=ctx2_ps, func=ACT_RELU)

                cf_ps = psX.tile([C, NT], F32, tag="cf")
                for k in range(NT):
                    nc.tensor.matmul(
                        cf_ps[:, k : k + 1], t_w3t[:, k * C : (k + 1) * C],
                        ctx2, start=True, stop=True,
                    )
                nc.scalar.copy(out=cfsb, in_=cf_ps)

            # ---------- phase C: dynamic filter + fusion conv ----------
            with tc.tile_pool(name="psBC", bufs=2, space="PSUM") as psBC, \
                 tc.tile_pool(name="psOut", bufs=2, space="PSUM") as psO, \
                 tc.tile_pool(name="pP", bufs=3) as pP, \
                 tc.tile_pool(name="pOsb", bufs=2) as pOsb:
                for t in range(NST):
                    out_ps = psO.tile([C, 2, 8, W], F32, tag="out_ps")
                    _absorb(nc, xs[0:1, t * STN : t * STN + 1],
                            out_ps[0:1, 0, 0, 0:1])
                    for h in range(2):
                        c0 = t * STN + h * 512
                        nc.tensor.matmul(
                            out_ps[:, h],
                            t_wfa,
                            xs[:, c0 : c0 + 512],
                            start=True,
                            stop=False,
                        )
                    for k in range(NT):
                        g = k % 2
                        bc_ps = psBC.tile([C, ROWS, W], F32, tag="bc")
                        if k == 0:
                            _absorb(nc, sfs[0:1, t * STN : t * STN + 1],
                                    bc_ps[0:1, 0, 0:1])
                        for h in range(2):
                            c0 = t * STN + h * 512
                            nc.tensor.matmul(
                                bc_ps[:, 8 * h : 8 * h + 8, :],
                                t_bct[32 * g : 32 * g + NT,
                                      k * C : (k + 1) * C],
                                sfs[32 * g : 32 * g + NT, c0 : c0 + 512],
                                start=True,
                                stop=True,
                                tile_position=(32 * g, 0),
                            )
                        dh, dw = divmod(k, 3)
                        p_sb = pP.tile([C, ROWS, W], F32R, tag="p")
                        nc.vector.scalar_tensor_tensor(
                            out=p_sb,
                            in0=bc_ps,
                            scalar=cfsb[:, k : k + 1],
                            in1=xpad[:, 16 * t + dh : 16 * t + dh + ROWS,
                                     dw : dw + W],
                            op0=ADD,
                            op1=MULT,
                        )
                        for h in range(2):
                            nc.tensor.matmul(
                                out_ps[:, h],
                                t_wfb,
                                p_sb[:, 8 * h : 8 * h + 8, :],
                                start=False,
                                stop=(k == NT - 1),
                            )
                    o_sb = pOsb.tile([C, 2, 8, W], F32, tag="osb")
                    nc.scalar.copy(out=o_sb, in_=out_ps)
                    nc.sync.dma_start(
                        out=ob[:, 16 * t : 16 * t + 16, :],
                        in_=o_sb.rearrange("c b r w -> c (b r) w"),
                    )
    _split_multiwaits(nc)
    return nc


def _prep_weights(static_w, w1, w2, w3, ws, wf):
    """Repack the tiny weights into the SBUF layouts the kernel expects."""
    f = np.float32
    sw = np.ascontiguousarray(static_w.reshape(C, NT), dtype=f)

    dsw = np.zeros((C, NT * C), dtype=f)
    for k in range(NT):
        dsw[np.arange(C), k * C + np.arange(C)] = sw[:, k]

    wsa = np.zeros((C, MREP), dtype=f)
    wsb = np.zeros((C, MREP), dtype=f)
    for g in range(4):
        for k in range(NT):
            wsa[:, 32 * g + k] = ws[k, :C]
            wsb[:, 32 * g + k] = ws[k, C:]

    bct = np.zeros((C, NT * C), dtype=f)
    for g in range(4):
        for k in range(NT):
            bct[32 * g + k, k * C : (k + 1) * C] = 1.0

    wfa = np.ascontiguousarray(wf[:, :C].T, dtype=f)
    wfb = np.ascontiguousarray(wf[:, C:].T, dtype=f)
    wr = round_f32r(
        np.concatenate([dsw, wsa, wsb, wfa, wfb, bct], axis=1)
    )
    wfp = np.concatenate(
        [np.ascontiguousarray(w1[:, :C].T, dtype=f),
         np.ascontiguousarray(w1[:, C:].T, dtype=f)], axis=1
    )
    w3t = np.ascontiguousarray(
        w3.reshape(C, NT, 64).transpose(2, 1, 0), dtype=f
    ).reshape(64, NT * C)
    wg64 = np.concatenate(
        [np.ascontiguousarray(w2.T, dtype=f), w3t], axis=1
    )
    wg = np.zeros((C, wg64.shape[1]), dtype=f)
    wg[:64] = wg64
    return np.concatenate([wr, wfp, wg], axis=1)


def make_in_maps(X2, Y2, static_w, w1, w2, w3, ws, wf):
    wpack = _prep_weights(
        np.asarray(static_w), np.asarray(w1), np.asarray(w2),
        np.asarray(w3), np.asarray(ws), np.asarray(wf),
    )
    X2 = np.asarray(X2)
    Y2 = np.asarray(Y2)
    xpad_all = np.zeros((B, C, PH, PW), dtype=np.float32)
    xpad_all[:, :, 1 : H + 1, 1 : W + 1] = X2
    xpad_all = round_f32r(xpad_all).reshape(B, C, PH * PW)
    y2_all = round_f32r(Y2.reshape(B, C, HW))
    in_maps = []
    for b in range(B):
        m = {"pk": np.ascontiguousarray(np.concatenate(
            [xpad_all[b], y2_all[b], wpack], axis=1))}
        in_maps.append(m)
    return in_maps


def get_nc():
    if "nc" not in _CACHE:
        _CACHE["nc"] = _build_bass()
    return _CACHE["nc"]


def kernel(X2, Y2, static_w, w1, w2, w3, ws, wf):
    nc = get_nc()
    in_maps = make_in_maps(
        np.asarray(X2), np.asarray(Y2), static_w, w1, w2, w3, ws, wf
    )
    res = run_bass_kernel_spmd(nc, in_maps, core_ids=list(range(B)))
    out = np.stack([r["ob"] for r in res.results]).astype(np.float32)
    return out
